# revision 3
# baseline (speedup 1.0000x reference)
"""Trainium2 Bass kernel for nn_CausalFullAttention_37821482009327.

Causal full attention (no softmax) with data-dependent complex relative
position decay, silu gating, and output projection.

Sharding: tensor-parallel over the 16 heads -> 2 heads per NeuronCore x 8.
Each core computes its heads' attention and a partial out-projection
(contraction over its 128-wide dim_inner slice); the host sums the 8
partials (the "all-reduce" happens at gather time).

Key layout decisions vs the earlier version:
- X is transposed on the HOST; the kernel DMAs X^T [D, N] directly, so
  no PE transposes / copies are spent building xT (was 128 matmuls).
- a-projection runs in f32r (1 cyc/row) instead of fp32 (4 cyc/row).
- silu(g) = g * sigmoid(g) so the scalar engine only ever needs the
  sqrt + sigmoid activation table sets (2 switches/chunk, not 3).
- v / attention-weight path in bf16 (1 cyc/row transposes, 2x casts).
- Separate PSUM tags per pipeline stage so the PE queue stays dense.

Shapes (hardcoded): B=1, N=2048, D=1024, H=16, Dh=64, Dc=32.
"""
import sys

sys.path.insert(0, "/opt/trn_rl_repo")

import numpy as np

import concourse.bass as bass
import concourse.tile as tile
from concourse import bacc, mybir
from concourse.bass_utils import run_bass_kernel_spmd
from concourse.masks import make_identity

F32 = mybir.dt.float32
F32R = mybir.dt.float32r  # TF32-class matmul fast path (1 cyc/row vs 4)
BF16 = mybir.dt.bfloat16

N = 2048
D = 1024
H_LOC = 2          # heads per core
DH = 64
DC = 32
NCORES = 8
EPS = 1e-10
SS_BF16 = True     # attention weights + v in bf16

NCH = N // 128     # 16 n-chunks of 128
DCH = D // 128     # 8 d-chunks of 128
NC4 = N // 512     # 4 n-chunks of 512

SSDT = BF16 if SS_BF16 else F32R


def _emit(nc):
    """Emit the per-core program (SPMD: same program, per-core weight data).

    Software pipeline over 512-row chunks c4=0..3 with DMA prefetch one
    round ahead of the a-projection, which itself runs one round ahead
    of the main chunk body. Causality means chunk c4's attention only
    consumes chunks <= c4.
    """
    XT = nc.dram_tensor("XT", [D, N], F32R, kind="ExternalInput")
    WQ = nc.dram_tensor("WQ", [D, 128], F32R, kind="ExternalInput")
    WK = nc.dram_tensor("WK", [D, 128], F32R, kind="ExternalInput")
    WA = nc.dram_tensor("WA", [D, 128], F32R, kind="ExternalInput")
    WV = nc.dram_tensor("WV", [D, 128], F32R, kind="ExternalInput")
    WG = nc.dram_tensor("WG", [D, 128], F32R, kind="ExternalInput")
    WO = nc.dram_tensor("WO", [128, D], F32R, kind="ExternalInput")
    BT = nc.dram_tensor("BT", [128, 8], F32, kind="ExternalInput")
    OUT = nc.dram_tensor("OUT", [D, N], F32, kind="ExternalOutput")

    with (
        tile.TileContext(nc) as tc,
        tc.tile_pool(name="pers", bufs=1) as pers,
        tc.tile_pool(name="ps", bufs=1, space="PSUM") as ps,
        tc.tile_pool(name="scan", bufs=2) as scan,
        tc.tile_pool(name="aep", bufs=2) as aep,
        tc.tile_pool(name="sse", bufs=4) as sse,
        tc.tile_pool(name="chk", bufs=2) as chk,
        tc.tile_pool(name="gte", bufs=2) as gte,
        tc.tile_pool(name="ote", bufs=3) as ote,
    ):
        # ---- persistent SBUF tensors ----
        identf = pers.tile([128, 128], F32, tag="identf")
        make_identity(nc, identf[:])
        identb = pers.tile([128, 128], BF16, tag="identb")
        make_identity(nc, identb[:])
        maskt = pers.tile([128, 4 * 512], F32, tag="maskt")
        for off in range(4):
            m = maskt[:, off * 512:(off + 1) * 512]
            nc.gpsimd.memset(m, 1.0)
            # keep (== leave 1.0) where f >= p + 128*off, else 0
            nc.gpsimd.affine_select(
                out=m, in_=m, compare_op=mybir.AluOpType.is_ge,
                fill=0.0, base=-128 * off, pattern=[[1, 512]],
                channel_multiplier=-1)

        xt = [pers.tile([128, N], F32R, tag=f"xt{dc}", name=f"xt{dc}")
              for dc in range(DCH)]
        kt_h = [pers.tile([64, N], F32R, tag=f"kt{h}", name=f"kt{h}")
                for h in range(H_LOC)]
        vb = pers.tile([128, N], SSDT, tag="vb")  # v natural [j_lo][jc*128+col]

        wa_t = pers.tile([128, D], F32R, tag="wa_t")
        for dc in range(DCH):
            nc.sync.dma_start(wa_t[:, dc * 128:(dc + 1) * 128],
                              WA[dc * 128:(dc + 1) * 128, :])
        wq_t = pers.tile([128, D], F32R, tag="wq_t")
        wk_t = pers.tile([128, D], F32R, tag="wk_t")
        wg_t = pers.tile([128, D], F32R, tag="wg_t")
        wv_t = pers.tile([128, D], F32R, tag="wv_t")
        wo_t = pers.tile([128, D], F32R, tag="wo_t")
        btile = pers.tile([128, 8], F32, tag="btile")
        for wt, WT in ((wv_t, WV), (wg_t, WG), (wq_t, WQ), (wk_t, WK)):
            nc.sync.dma_start(
                wt[:].rearrange("p (dc c) -> p dc c", dc=DCH),
                WT[:].rearrange("(dc p) c -> p dc c", p=128))
        nc.sync.dma_start(wo_t[:], WO[:])
        nc.sync.dma_start(btile[:], BT[:])
        wq_sb = [wq_t[:, dc * 128:(dc + 1) * 128] for dc in range(DCH)]
        wk_sb = [wk_t[:, dc * 128:(dc + 1) * 128] for dc in range(DCH)]
        wg_sb = [wg_t[:, dc * 128:(dc + 1) * 128] for dc in range(DCH)]
        wv_sb = [wv_t[:, dc * 128:(dc + 1) * 128] for dc in range(DCH)]
        wa_sb = [wa_t[:, dc * 128:(dc + 1) * 128] for dc in range(DCH)]
        wo_sb = [wo_t[:, ji * 128:(ji + 1) * 128] for ji in range(DCH)]

        state = {}

        def dma_x(c4):
            ns = slice(c4 * 512, (c4 + 1) * 512)
            for dc in range(DCH):
                nc.sync.dma_start(xt[dc][:, ns],
                                  XT[dc * 128:(dc + 1) * 128, ns])

        def aproj(c4):
            ns = slice(c4 * 512, (c4 + 1) * 512)
            pa = ps.tile([128, 512], F32, tag="pa", bufs=1, name="pa")
            for dc in range(DCH):
                nc.tensor.matmul(pa[:], wa_sb[dc], xt[dc][:, ns],
                                 start=(dc == 0), stop=(dc == DCH - 1))
            state[c4] = pa

        def body(c4):
            ns = slice(c4 * 512, (c4 + 1) * 512)
            pa = state.pop(c4)

            # ---- 1. aT -> natural layout for the d-scan ----
            at_sb = aep.tile([128, 512], F32, tag="at_sb", name="at_sb")
            nc.scalar.copy(at_sb[:], pa[:])
            pan = ps.tile([128, 512], F32, tag="ptr", bufs=1, name="pan")
            for s in range(4):
                nc.tensor.transpose(
                    pan[:, s * 128:(s + 1) * 128],
                    at_sb[:, s * 128:(s + 1) * 128], identf[:])
            # scan chunk buffers [128, 256] = [s 4][h 2][d 32]
            reA = scan.tile([128, 256], F32, tag="reA", name="reA")
            imA = scan.tile([128, 256], F32, tag="imA", name="imA")
            reB = scan.tile([128, 256], F32, tag="reB", name="reB")
            imB = scan.tile([128, 256], F32, tag="imB", name="imB")
            t1 = scan.tile([128, 256], F32, tag="t1", name="t1", bufs=1)
            t2 = scan.tile([128, 256], F32, tag="t2", name="t2", bufs=1)
            src_re = pan[:].rearrange(
                "p (s c h d) -> p s c h d", s=4, c=2, h=2)[:, :, 0]
            src_im = pan[:].rearrange(
                "p (s c h d) -> p s c h d", s=4, c=2, h=2)[:, :, 1]
            nc.vector.tensor_copy(
                reA[:].rearrange("p (s h d) -> p s h d", s=4, h=2), src_re)
            nc.vector.tensor_copy(
                imA[:].rearrange("p (s h d) -> p s h d", s=4, h=2), src_im)

            # ---- 2. pointwise: ac = a * sigmoid(|a|)/|a| ----
            nc.vector.tensor_mul(t1[:], reA[:], reA[:])
            nc.vector.tensor_mul(t2[:], imA[:], imA[:])
            nc.vector.tensor_add(t1[:], t1[:], t2[:])          # |a|^2
            nc.scalar.activation(t2[:], t1[:], mybir.ActivationFunctionType.Sqrt)
            nc.vector.reciprocal_approx_fast(t1[:], t2[:])     # 1/|a|
            nc.scalar.activation(t2[:], t2[:],
                                 mybir.ActivationFunctionType.Sigmoid)
            nc.vector.tensor_mul(t1[:], t1[:], t2[:])          # sig(|a|)/|a|
            nc.vector.tensor_mul(reA[:], reA[:], t1[:])
            nc.vector.tensor_mul(imA[:], imA[:], t1[:])

            # ---- 3. doubling scan (complex cumprod over d) ----
            # prefix copies ride the scalar engine (Copy: no table switch)
            def blk(buf, lo, hi):
                return buf[:].rearrange("p (b w) -> p b w", w=32)[:, :, lo:hi]

            sre, sim_, dre, dim_ = reA, imA, reB, imB
            for si, s in enumerate((1, 2, 4, 8, 16)):
                w = 32 - s
                r0, i0 = blk(sre, s, 32), blk(sim_, s, 32)
                rs, is_ = blk(sre, 0, w), blk(sim_, 0, w)
                rd, id_ = blk(dre, s, 32), blk(dim_, s, 32)
                tt1, tt2 = blk(t1, 0, w), blk(t2, 0, w)
                nc.scalar.copy(blk(dre, 0, s), blk(sre, 0, s))
                nc.vector.tensor_mul(tt1, r0, rs)
                nc.vector.tensor_mul(tt2, i0, is_)
                nc.vector.tensor_sub(rd, tt1, tt2)
                if si < 4:
                    nc.scalar.copy(blk(dim_, 0, s), blk(sim_, 0, s))
                    nc.vector.tensor_mul(tt1, r0, is_)
                    nc.vector.tensor_mul(tt2, i0, rs)
                    nc.vector.tensor_add(id_, tt1, tt2)
                sre, dre = dre, sre
                sim_, dim_ = dim_, sim_
            # final real part is in reB

            # ---- 4. v projection (f32r -> bf16 natural) ----
            pv = ps.tile([128, 512], F32, tag="pjA", bufs=1, name="pv")
            for dc in range(DCH):
                nc.tensor.matmul(pv[:], wv_sb[dc], xt[dc][:, ns],
                                 start=(dc == 0), stop=(dc == DCH - 1))
            vtile = gte.tile([128, 512], SSDT, tag="vt", name="vt")
            nc.vector.tensor_copy(vtile[:], pv[:])
            pvn = ps.tile([128, 512], SSDT, tag="ptr", bufs=1, name="pvn")
            ident_v = identb if SS_BF16 else identf
            for s in range(4):
                nc.tensor.transpose(
                    pvn[:, s * 128:(s + 1) * 128],
                    vtile[:, s * 128:(s + 1) * 128], ident_v[:])
            nc.vector.tensor_copy(vb[:, ns], pvn[:])

            # ---- 5. g projection; silu(g) = g * sigmoid(g) ----
            pg = ps.tile([128, 512], F32, tag="pjB", bufs=1, name="pg")
            for dc in range(DCH):
                nc.tensor.matmul(pg[:], wg_sb[dc], xt[dc][:, ns],
                                 start=(dc == 0), stop=(dc == DCH - 1))
            gsig = chk.tile([128, 512], F32, tag="gsig", name="gsig")
            nc.scalar.activation(gsig[:], pg[:],
                                 mybir.ActivationFunctionType.Sigmoid)
            gsC = chk.tile([128, 512], F32, tag="gsC", name="gsC")
            nc.vector.tensor_mul(gsC[:], gsig[:], pg[:])

            # ---- 6. acr: clip, expand pairs, transpose; 1/acr ----
            acrC = chk.tile([128, 512], F32, tag="acrC", name="acrC")
            krC = chk.tile([128, 512], F32, tag="krC", name="krC")
            pae = ps.tile([128, 512], F32, tag="ptr", bufs=1, name="pae")
            for si in range(4):
                ae = aep.tile([128, 128], F32, tag="ae", bufs=2)
                src = reB[:, si * 64:(si + 1) * 64].rearrange(
                    "p (h d) -> p h d", h=2)
                for c in range(2):
                    dst = ae[:].rearrange(
                        "p (h d two) -> p h d two", h=2, two=2)[:, :, :, c]
                    nc.vector.tensor_scalar_max(dst, src, EPS)
                nc.tensor.transpose(pae[:, si * 128:(si + 1) * 128],
                                    ae[:], identf[:])
            nc.vector.tensor_copy(acrC[:], pae[:])
            nc.vector.reciprocal_approx_fast(krC[:], acrC[:])

            # ---- 7. q/k projections + decay scaling (f32r) ----
            pq = ps.tile([128, 512], F32, tag="pjA", bufs=1, name="pq")
            for dc in range(DCH):
                nc.tensor.matmul(pq[:], wq_sb[dc], xt[dc][:, ns],
                                 start=(dc == 0), stop=(dc == DCH - 1))
            qt_c = [chk.tile([64, 512], F32R, tag=f"qt{h}", name=f"qt{h}")
                    for h in range(H_LOC)]
            for h in range(H_LOC):
                hp = slice(h * 64, (h + 1) * 64)
                nc.vector.tensor_mul(qt_c[h][:], pq[hp, :], acrC[hp, :])
            pk = ps.tile([128, 512], F32, tag="pjB", bufs=1, name="pk")
            for dc in range(DCH):
                nc.tensor.matmul(pk[:], wk_sb[dc], xt[dc][:, ns],
                                 start=(dc == 0), stop=(dc == DCH - 1))
            for h in range(H_LOC):
                hp = slice(h * 64, (h + 1) * 64)
                nc.vector.tensor_mul(kt_h[h][:, ns], pk[hp, :], krC[hp, :])

            # ---- 8. causal attention + gating + partial out-projection ----
            pout = ps.tile([128, 512], F32, tag="pout", bufs=1, name="pout")
            njc = 4 * (c4 + 1)

            def av_mm(ss_pair, jc):
                for h in range(H_LOC):
                    nc.tensor.matmul(
                        pout[h * 64:(h + 1) * 64, :],
                        vb[:, jc * 128 + h * 64: jc * 128 + h * 64 + 64],
                        ss_pair[h][:], start=(jc == 0), stop=(jc == njc - 1),
                        skip_group_check=True)

            pend = []
            for jc in range(njc):
                psims = []
                for h in range(H_LOC):
                    psim = ps.tile([128, 512], F32, tag="psim", name="psim",
                                   bufs=3)
                    nc.tensor.matmul(
                        psim[:], kt_h[h][:, jc * 128:(jc + 1) * 128],
                        qt_c[h][:], start=True, stop=True)
                    psims.append(psim)
                sss = []
                off = jc - 4 * c4
                for h in range(H_LOC):
                    ss = sse.tile([128, 512], SSDT, tag="ss", name="ss",
                                  bufs=8)
                    if off >= 0 and h == 0:
                        nc.vector.tensor_mul(
                            ss[:], psims[h][:],
                            maskt[:, off * 512:(off + 1) * 512])
                    elif off >= 0:
                        tmp = sse.tile([128, 512], SSDT, tag="sstmp",
                                       name="sstmp", bufs=2)
                        nc.scalar.copy(tmp[:], psims[h][:])
                        nc.gpsimd.affine_select(
                            out=ss[:], in_=tmp[:],
                            compare_op=mybir.AluOpType.is_ge,
                            fill=0.0, base=-128 * off, pattern=[[1, 512]],
                            channel_multiplier=-1)
                    elif h == 0:
                        nc.scalar.copy(ss[:], psims[h][:])
                    else:
                        nc.vector.tensor_copy(ss[:], psims[h][:])
                    sss.append(ss)
                pend.append((sss, jc))
                if len(pend) > 2:
                    av_mm(*pend.pop(0))
            for it in pend:
                av_mm(*it)
            gt_ = gte.tile([128, 512], F32R, tag="gt")
            nc.vector.tensor_mul(gt_[:], pout[:], gsC[:])
            for ji in range(DCH):
                poj = ps.tile([128, 512], F32, tag="psim", bufs=3,
                              name="poj")
                nc.tensor.matmul(poj[:], wo_sb[ji], gt_[:],
                                 start=True, stop=True)
                ot = ote.tile([128, 512], F32, tag="ot", bufs=3)
                if ji % 2 == 0:
                    nc.scalar.activation(
                        ot[:], poj[:],
                        mybir.ActivationFunctionType.Identity,
                        bias=btile[:, ji:ji + 1])
                else:
                    nc.vector.tensor_scalar_add(
                        ot[:], poj[:], btile[:, ji:ji + 1])
                eng = nc.sync if ji % 2 == 0 else nc.scalar
                eng.dma_start(OUT[ji * 128:(ji + 1) * 128, ns], ot[:])

        dma_x(0)
        aproj(0)
        dma_x(1)
        for c4 in range(NC4):
            if c4 + 2 < NC4:
                dma_x(c4 + 2)
            if c4 + 1 < NC4:
                aproj(c4 + 1)
            body(c4)
    nc.finalize()
    return nc


_NC_CACHE = []


def _get_nc():
    if not _NC_CACHE:
        nc = bacc.Bacc("TRN2", target_bir_lowering=False, debug=False)
        _emit(nc)
        _NC_CACHE.append(nc)
    return _NC_CACHE[0]


def _shard_inputs(x, W_qkv, W_a, W_g, W_out, b_out):
    x2 = np.asarray(x, np.float32).reshape(N, D)
    xT = np.ascontiguousarray(x2.T)  # [D, N] — shared across cores
    W_qkv = np.asarray(W_qkv, np.float32)
    W_a = np.asarray(W_a, np.float32)
    W_g = np.asarray(W_g, np.float32)
    W_out = np.asarray(W_out, np.float32)
    b_out = np.asarray(b_out, np.float32)

    # W_a column permutation: within a core's 128 cols, source col
    # h*64 + 2d + c  ->  dest col c*64 + h*32 + d
    perm = np.empty(128, np.int64)
    for c in range(2):
        for h in range(2):
            for d in range(DC):
                perm[c * 64 + h * 32 + d] = h * 64 + 2 * d + c

    in_maps = []
    for r in range(NCORES):
        cs = r * 128
        wq = np.ascontiguousarray(W_qkv[:, cs:cs + 128] * np.float32(DH ** -0.5))
        wk = np.ascontiguousarray(W_qkv[:, D + cs:D + cs + 128])
        wv = np.ascontiguousarray(W_qkv[:, 2 * D + cs:2 * D + cs + 128])
        wa = np.ascontiguousarray(W_a[:, cs:cs + 128][:, perm])
        wg = np.ascontiguousarray(W_g[:, cs:cs + 128])
        wo = np.ascontiguousarray(W_out[cs:cs + 128, :])
        if r == 0:
            bt = np.ascontiguousarray(b_out.reshape(8, 128).T)
        else:
            bt = np.zeros((128, 8), np.float32)
        in_maps.append({
            "XT": xT, "WQ": wq, "WK": wk, "WA": wa, "WV": wv, "WG": wg,
            "WO": wo, "BT": bt,
        })
    return in_maps


def _unshard(results):
    outT = np.zeros((D, N), np.float32)
    for r in results:
        outT += r["OUT"]
    return np.ascontiguousarray(outT.T).reshape(1, N, D)


def run(trace=False, **inputs):
    nc = _get_nc()
    in_maps = _shard_inputs(**inputs)
    res = run_bass_kernel_spmd(nc, in_maps, core_ids=list(range(NCORES)),
                               trace=trace)
    return _unshard(res.results), res


def kernel(**inputs) -> np.ndarray:
    out, _ = run(trace=False, **inputs)
    return out


# revision 5
# speedup vs baseline: 1.0203x; 1.0203x over previous
"""Trainium2 Bass kernel for nn_CausalFullAttention_37821482009327.

Causal full attention (no softmax) with data-dependent complex relative
position decay, silu gating, and output projection.

Sharding: tensor-parallel over the 16 heads -> 2 heads per NeuronCore x 8.
Each core computes its heads' attention and a partial out-projection
(contraction over its 128-wide dim_inner slice); the host sums the 8
partials (the "all-reduce" happens at gather time).

Layout / schedule decisions:
- X is transposed on the HOST; the kernel DMAs X^T [D, N] directly, so
  no PE transposes / copies are spent building xT.
- The a-projection runs in true fp32 (bitcast of the f32r x tiles) —
  the d-cumprod amplifies tf32 rounding past the error budget.
- silu(g) = g * sigmoid(g) so the scalar engine only needs the
  sqrt + sigmoid activation table sets (2 switches/chunk, not 3).
- v / attention-weight path in bf16.
- Software pipeline: round(c4) emits next chunk's a-projection first,
  then this chunk's projections + attention + output, and finally the
  next chunk's scan chain (vector/scalar) so it overlaps the heavy PE
  phases of the current chunk. The acr tail (PE transposes) lands at
  the top of the next round behind ~28 matmuls of buffer.

Shapes (hardcoded): B=1, N=2048, D=1024, H=16, Dh=64, Dc=32.
"""
import sys

sys.path.insert(0, "/opt/trn_rl_repo")

import numpy as np

import concourse.bass as bass
import concourse.tile as tile
from concourse import bacc, mybir
from concourse.bass_utils import run_bass_kernel_spmd
from concourse.masks import make_identity

F32 = mybir.dt.float32
F32R = mybir.dt.float32r  # TF32-class matmul fast path (1 cyc/row vs 4)
BF16 = mybir.dt.bfloat16

N = 2048
D = 1024
H_LOC = 2          # heads per core
DH = 64
DC = 32
NCORES = 8
EPS = 1e-10
SS_BF16 = True     # attention weights + v in bf16

DCH = D // 128     # 8 d-chunks of 128
NC4 = N // 512     # 4 n-chunks of 512

SSDT = BF16 if SS_BF16 else F32R
AF = mybir.ActivationFunctionType


def _emit(nc):
    XT = nc.dram_tensor("XT", [D, N], F32R, kind="ExternalInput")
    WQ = nc.dram_tensor("WQ", [D, 128], F32R, kind="ExternalInput")
    WK = nc.dram_tensor("WK", [D, 128], F32R, kind="ExternalInput")
    WA = nc.dram_tensor("WA", [D, 128], F32, kind="ExternalInput")
    WV = nc.dram_tensor("WV", [D, 128], F32R, kind="ExternalInput")
    WG = nc.dram_tensor("WG", [D, 128], F32R, kind="ExternalInput")
    WO = nc.dram_tensor("WO", [128, D], F32R, kind="ExternalInput")
    BT = nc.dram_tensor("BT", [128, 8], F32, kind="ExternalInput")
    OUT = nc.dram_tensor("OUT", [D, N], F32, kind="ExternalOutput")

    with (
        tile.TileContext(nc) as tc,
        tc.tile_pool(name="pers", bufs=1) as pers,
        tc.tile_pool(name="ps", bufs=1, space="PSUM") as ps,
        tc.tile_pool(name="scan", bufs=2) as scan,
        tc.tile_pool(name="aep", bufs=2) as aep,
        tc.tile_pool(name="sse", bufs=4) as sse,
        tc.tile_pool(name="chk", bufs=2) as chk,
        tc.tile_pool(name="gte", bufs=2) as gte,
        tc.tile_pool(name="ote", bufs=3) as ote,
    ):
        # ---- persistent SBUF tensors ----
        identf = pers.tile([128, 128], F32, tag="identf")
        make_identity(nc, identf[:])
        identb = pers.tile([128, 128], BF16, tag="identb")
        make_identity(nc, identb[:])
        maskt = pers.tile([128, 4 * 512], F32, tag="maskt")
        for off in range(4):
            m = maskt[:, off * 512:(off + 1) * 512]
            nc.gpsimd.memset(m, 1.0)
            # keep (== leave 1.0) where f >= p + 128*off, else 0
            nc.gpsimd.affine_select(
                out=m, in_=m, compare_op=mybir.AluOpType.is_ge,
                fill=0.0, base=-128 * off, pattern=[[1, 512]],
                channel_multiplier=-1)

        xt = [pers.tile([128, N], F32R, tag=f"xt{dc}", name=f"xt{dc}")
              for dc in range(DCH)]
        kt_h = [pers.tile([64, N], F32R, tag=f"kt{h}", name=f"kt{h}")
                for h in range(H_LOC)]
        vb = pers.tile([128, N], SSDT, tag="vb")  # v natural [j_lo][jc*128+col]

        wa_t = pers.tile([128, D], F32, tag="wa_t")
        for dc in range(DCH):
            nc.sync.dma_start(wa_t[:, dc * 128:(dc + 1) * 128],
                              WA[dc * 128:(dc + 1) * 128, :])
        wq_t = pers.tile([128, D], F32R, tag="wq_t")
        wk_t = pers.tile([128, D], F32R, tag="wk_t")
        wg_t = pers.tile([128, D], F32R, tag="wg_t")
        wv_t = pers.tile([128, D], F32R, tag="wv_t")
        wo_t = pers.tile([128, D], F32R, tag="wo_t")
        btile = pers.tile([128, 8], F32, tag="btile")
        for wt, WT in ((wv_t, WV), (wg_t, WG), (wq_t, WQ), (wk_t, WK)):
            nc.sync.dma_start(
                wt[:].rearrange("p (dc c) -> p dc c", dc=DCH),
                WT[:].rearrange("(dc p) c -> p dc c", p=128))
        nc.sync.dma_start(wo_t[:], WO[:])
        nc.sync.dma_start(btile[:], BT[:])
        wq_sb = [wq_t[:, dc * 128:(dc + 1) * 128] for dc in range(DCH)]
        wk_sb = [wk_t[:, dc * 128:(dc + 1) * 128] for dc in range(DCH)]
        wg_sb = [wg_t[:, dc * 128:(dc + 1) * 128] for dc in range(DCH)]
        wv_sb = [wv_t[:, dc * 128:(dc + 1) * 128] for dc in range(DCH)]
        wa_sb = [wa_t[:, dc * 128:(dc + 1) * 128] for dc in range(DCH)]
        wo_sb = [wo_t[:, ji * 128:(ji + 1) * 128] for ji in range(DCH)]

        st = {}   # per-chunk cross-round tiles

        def dma_x(c4):
            ns = slice(c4 * 512, (c4 + 1) * 512)
            for dc in range(DCH):
                nc.sync.dma_start(xt[dc][:, ns],
                                  XT[dc * 128:(dc + 1) * 128, ns])

        def aproj(c4):
            # fp32 matmul (4 cyc/row): full precision for the d-cumprod
            ns = slice(c4 * 512, (c4 + 1) * 512)
            pa = ps.tile([128, 512], F32, tag="pa", bufs=1, name="pa")
            for dc in range(DCH):
                nc.tensor.matmul(pa[:], wa_sb[dc],
                                 xt[dc][:, ns].bitcast(F32),
                                 start=(dc == 0), stop=(dc == DCH - 1))
            st[("pa", c4)] = pa

        def chain_ab(c4):
            """aT -> natural, pointwise decay factor, doubling scan.

            Scalar: 1 copy + sqrt + sigmoid (2 table switches) + scan
            prefix copies. Vector: the rest. Ends with reB = Re(cumprod).
            """
            pa = st.pop(("pa", c4))
            at_sb = aep.tile([128, 512], F32, tag="at_sb", name="at_sb")
            nc.scalar.copy(at_sb[:], pa[:])
            pan = ps.tile([128, 512], F32, tag="ptr", bufs=1, name="pan")
            for s in range(4):
                nc.tensor.transpose(
                    pan[:, s * 128:(s + 1) * 128],
                    at_sb[:, s * 128:(s + 1) * 128], identf[:])
            # scan chunk buffers [128, 256] = [s 4][h 2][d 32]
            reA = scan.tile([128, 256], F32, tag="reA", name="reA")
            imA = scan.tile([128, 256], F32, tag="imA", name="imA")
            reB = scan.tile([128, 256], F32, tag="reB", name="reB")
            imB = scan.tile([128, 256], F32, tag="imB", name="imB")
            t1 = scan.tile([128, 256], F32, tag="t1", name="t1", bufs=1)
            t2 = scan.tile([128, 256], F32, tag="t2", name="t2", bufs=1)
            src_re = pan[:].rearrange(
                "p (s c h d) -> p s c h d", s=4, c=2, h=2)[:, :, 0]
            src_im = pan[:].rearrange(
                "p (s c h d) -> p s c h d", s=4, c=2, h=2)[:, :, 1]
            nc.vector.tensor_copy(
                reA[:].rearrange("p (s h d) -> p s h d", s=4, h=2), src_re)
            nc.vector.tensor_copy(
                imA[:].rearrange("p (s h d) -> p s h d", s=4, h=2), src_im)

            # pointwise: ac = a * sigmoid(|a|)/|a|
            nc.vector.tensor_mul(t1[:], reA[:], reA[:])
            nc.vector.tensor_mul(t2[:], imA[:], imA[:])
            nc.vector.tensor_add(t1[:], t1[:], t2[:])          # |a|^2
            nc.scalar.activation(t2[:], t1[:], AF.Sqrt)
            nc.vector.reciprocal_approx_fast(t1[:], t2[:])     # 1/|a|
            nc.scalar.activation(t2[:], t2[:], AF.Sigmoid)
            nc.vector.tensor_mul(t1[:], t1[:], t2[:])          # sig(|a|)/|a|
            nc.vector.tensor_mul(reA[:], reA[:], t1[:])
            nc.vector.tensor_mul(imA[:], imA[:], t1[:])

            # doubling scan (complex cumprod over d); prefix copies ride
            # the scalar engine (Copy: no table switch)
            def blk(buf, lo, hi):
                return buf[:].rearrange("p (b w) -> p b w", w=32)[:, :, lo:hi]

            sre, sim_, dre, dim_ = reA, imA, reB, imB
            for si, s in enumerate((1, 2, 4, 8, 16)):
                w = 32 - s
                r0, i0 = blk(sre, s, 32), blk(sim_, s, 32)
                rs, is_ = blk(sre, 0, w), blk(sim_, 0, w)
                rd, id_ = blk(dre, s, 32), blk(dim_, s, 32)
                tt1, tt2 = blk(t1, 0, w), blk(t2, 0, w)
                nc.scalar.copy(blk(dre, 0, s), blk(sre, 0, s))
                nc.vector.tensor_mul(tt1, r0, rs)
                nc.vector.tensor_mul(tt2, i0, is_)
                nc.vector.tensor_sub(rd, tt1, tt2)
                if si < 4:
                    nc.scalar.copy(blk(dim_, 0, s), blk(sim_, 0, s))
                    nc.vector.tensor_mul(tt1, r0, is_)
                    nc.vector.tensor_mul(tt2, i0, rs)
                    nc.vector.tensor_add(id_, tt1, tt2)
                sre, dre = dre, sre
                sim_, dim_ = dim_, sim_
            st[("reB", c4)] = reB  # final real part

        def chain_c(c4):
            """acr: clip, expand pairs, transpose to T layout; 1/acr."""
            reB = st.pop(("reB", c4))
            acrC = chk.tile([128, 512], F32, tag="acrC", name="acrC")
            krC = chk.tile([128, 512], F32, tag="krC", name="krC")
            pae = ps.tile([128, 512], F32, tag="ptr", bufs=1, name="pae")
            for si in range(4):
                ae = aep.tile([128, 128], F32, tag="ae", bufs=2)
                src = reB[:, si * 64:(si + 1) * 64].rearrange(
                    "p (h d) -> p h d", h=2)
                for c in range(2):
                    dst = ae[:].rearrange(
                        "p (h d two) -> p h d two", h=2, two=2)[:, :, :, c]
                    nc.vector.tensor_scalar_max(dst, src, EPS)
                nc.tensor.transpose(pae[:, si * 128:(si + 1) * 128],
                                    ae[:], identf[:])
            nc.vector.tensor_copy(acrC[:], pae[:])
            nc.vector.reciprocal_approx_fast(krC[:], acrC[:])
            st[("acr", c4)] = (acrC, krC)

        def body(c4):
            ns = slice(c4 * 512, (c4 + 1) * 512)

            # ---- v projection (f32r -> bf16 natural) ----
            pv = ps.tile([128, 512], F32, tag="pjA", bufs=1, name="pv")
            for dc in range(DCH):
                nc.tensor.matmul(pv[:], wv_sb[dc], xt[dc][:, ns],
                                 start=(dc == 0), stop=(dc == DCH - 1))
            vtile = gte.tile([128, 512], SSDT, tag="vt", name="vt")
            nc.vector.tensor_copy(vtile[:], pv[:])
            pvn = ps.tile([128, 512], SSDT, tag="ptr", bufs=1, name="pvn")
            ident_v = identb if SS_BF16 else identf
            for s in range(4):
                nc.tensor.transpose(
                    pvn[:, s * 128:(s + 1) * 128],
                    vtile[:, s * 128:(s + 1) * 128], ident_v[:])
            nc.vector.tensor_copy(vb[:, ns], pvn[:])

            # ---- g projection; silu(g) = g * sigmoid(g) ----
            pg = ps.tile([128, 512], F32, tag="pjB", bufs=1, name="pg")
            for dc in range(DCH):
                nc.tensor.matmul(pg[:], wg_sb[dc], xt[dc][:, ns],
                                 start=(dc == 0), stop=(dc == DCH - 1))
            gsig = chk.tile([128, 512], F32, tag="gsig", name="gsig")
            nc.scalar.activation(gsig[:], pg[:], AF.Sigmoid)
            gsC = chk.tile([128, 512], F32, tag="gsC", name="gsC")
            nc.vector.tensor_mul(gsC[:], gsig[:], pg[:])

            # ---- acr tail for this chunk (PE buffered by v/g above) ----
            chain_c(c4)
            acrC, krC = st.pop(("acr", c4))

            # ---- q/k projections + decay scaling (f32r) ----
            pq = ps.tile([128, 512], F32, tag="pjA", bufs=1, name="pq")
            for dc in range(DCH):
                nc.tensor.matmul(pq[:], wq_sb[dc], xt[dc][:, ns],
                                 start=(dc == 0), stop=(dc == DCH - 1))
            qt_c = [chk.tile([64, 512], F32R, tag=f"qt{h}", name=f"qt{h}")
                    for h in range(H_LOC)]
            for h in range(H_LOC):
                hp = slice(h * 64, (h + 1) * 64)
                nc.vector.tensor_mul(qt_c[h][:], pq[hp, :], acrC[hp, :])
            pk = ps.tile([128, 512], F32, tag="pjB", bufs=1, name="pk")
            for dc in range(DCH):
                nc.tensor.matmul(pk[:], wk_sb[dc], xt[dc][:, ns],
                                 start=(dc == 0), stop=(dc == DCH - 1))
            for h in range(H_LOC):
                hp = slice(h * 64, (h + 1) * 64)
                nc.vector.tensor_mul(kt_h[h][:, ns], pk[hp, :], krC[hp, :])

            # ---- causal attention ----
            pout = ps.tile([128, 512], F32, tag="pout", bufs=1, name="pout")
            njc = 4 * (c4 + 1)

            def av_mm(ss_pair, jc):
                for h in range(H_LOC):
                    nc.tensor.matmul(
                        pout[h * 64:(h + 1) * 64, :],
                        vb[:, jc * 128 + h * 64: jc * 128 + h * 64 + 64],
                        ss_pair[h][:], start=(jc == 0), stop=(jc == njc - 1),
                        skip_group_check=True)

            pend = []
            for jc in range(njc):
                psims = []
                for h in range(H_LOC):
                    psim = ps.tile([128, 512], F32, tag="psim", name="psim",
                                   bufs=3)
                    nc.tensor.matmul(
                        psim[:], kt_h[h][:, jc * 128:(jc + 1) * 128],
                        qt_c[h][:], start=True, stop=True)
                    psims.append(psim)
                sss = []
                off = jc - 4 * c4
                for h in range(H_LOC):
                    ss = sse.tile([128, 512], SSDT, tag="ss", name="ss",
                                  bufs=8)
                    if h == 0:
                        if off >= 0:
                            nc.vector.tensor_mul(
                                ss[:], psims[h][:],
                                maskt[:, off * 512:(off + 1) * 512])
                        else:
                            nc.vector.tensor_copy(ss[:], psims[h][:])
                    else:
                        if off >= 0:
                            tmp = sse.tile([128, 512], SSDT, tag="sstmp",
                                           name="sstmp", bufs=2)
                            nc.scalar.copy(tmp[:], psims[h][:])
                            nc.gpsimd.affine_select(
                                out=ss[:], in_=tmp[:],
                                compare_op=mybir.AluOpType.is_ge,
                                fill=0.0, base=-128 * off,
                                pattern=[[1, 512]], channel_multiplier=-1)
                        else:
                            nc.scalar.copy(ss[:], psims[h][:])
                    sss.append(ss)
                pend.append((sss, jc))
                if len(pend) > 2:
                    av_mm(*pend.pop(0))
            for it in pend:
                av_mm(*it)

            # ---- gating + partial out-projection ----
            gt_ = gte.tile([128, 512], F32R, tag="gt")
            nc.vector.tensor_mul(gt_[:], pout[:], gsC[:])
            for ji in range(DCH):
                poj = ps.tile([128, 512], F32, tag="psim", bufs=3,
                              name="poj")
                nc.tensor.matmul(poj[:], wo_sb[ji], gt_[:],
                                 start=True, stop=True)
                ot = ote.tile([128, 512], F32, tag="ot", bufs=3)
                if ji % 2 == 0:
                    nc.scalar.activation(ot[:], poj[:], AF.Identity,
                                         bias=btile[:, ji:ji + 1])
                else:
                    nc.vector.tensor_scalar_add(
                        ot[:], poj[:], btile[:, ji:ji + 1])
                eng = nc.sync if ji % 2 == 0 else nc.scalar
                eng.dma_start(OUT[ji * 128:(ji + 1) * 128, ns], ot[:])

        # ---- schedule ----
        dma_x(0)
        aproj(0)
        chain_ab(0)
        dma_x(1)
        for c4 in range(NC4):
            if c4 + 1 < NC4:
                aproj(c4 + 1)
            if c4 + 2 < NC4:
                dma_x(c4 + 2)
            body(c4)
            if c4 + 1 < NC4:
                chain_ab(c4 + 1)
    nc.finalize()
    return nc


_NC_CACHE = []


def _get_nc():
    if not _NC_CACHE:
        nc = bacc.Bacc("TRN2", target_bir_lowering=False, debug=False)
        _emit(nc)
        _NC_CACHE.append(nc)
    return _NC_CACHE[0]


def _shard_inputs(x, W_qkv, W_a, W_g, W_out, b_out):
    x2 = np.asarray(x, np.float32).reshape(N, D)
    xT = np.ascontiguousarray(x2.T)  # [D, N] — shared across cores
    W_qkv = np.asarray(W_qkv, np.float32)
    W_a = np.asarray(W_a, np.float32)
    W_g = np.asarray(W_g, np.float32)
    W_out = np.asarray(W_out, np.float32)
    b_out = np.asarray(b_out, np.float32)

    # W_a column permutation: within a core's 128 cols, source col
    # h*64 + 2d + c  ->  dest col c*64 + h*32 + d
    perm = np.empty(128, np.int64)
    for c in range(2):
        for h in range(2):
            for d in range(DC):
                perm[c * 64 + h * 32 + d] = h * 64 + 2 * d + c

    in_maps = []
    for r in range(NCORES):
        cs = r * 128
        wq = np.ascontiguousarray(W_qkv[:, cs:cs + 128] * np.float32(DH ** -0.5))
        wk = np.ascontiguousarray(W_qkv[:, D + cs:D + cs + 128])
        wv = np.ascontiguousarray(W_qkv[:, 2 * D + cs:2 * D + cs + 128])
        wa = np.ascontiguousarray(W_a[:, cs:cs + 128][:, perm])
        wg = np.ascontiguousarray(W_g[:, cs:cs + 128])
        wo = np.ascontiguousarray(W_out[cs:cs + 128, :])
        if r == 0:
            bt = np.ascontiguousarray(b_out.reshape(8, 128).T)
        else:
            bt = np.zeros((128, 8), np.float32)
        in_maps.append({
            "XT": xT, "WQ": wq, "WK": wk, "WA": wa, "WV": wv, "WG": wg,
            "WO": wo, "BT": bt,
        })
    return in_maps


def _unshard(results):
    outT = np.zeros((D, N), np.float32)
    for r in results:
        outT += r["OUT"]
    return np.ascontiguousarray(outT.T).reshape(1, N, D)


def run(trace=False, **inputs):
    nc = _get_nc()
    in_maps = _shard_inputs(**inputs)
    res = run_bass_kernel_spmd(nc, in_maps, core_ids=list(range(NCORES)),
                               trace=trace)
    return _unshard(res.results), res


def kernel(**inputs) -> np.ndarray:
    out, _ = run(trace=False, **inputs)
    return out
